# revision 5
# baseline (speedup 1.0000x reference)
"""Multi-head causal self-attention (B=2, T=4096, C=512, H=8) on 8 trn2 cores.

Sharding: 16 (batch, head) pairs -> 2 heads per core. Core c handles batch
c//4, heads {2*(c%4), 2*(c%4)+1}. Each core computes its heads' Q/K/V
projections from host-pre-transposed activations, runs causal flash
attention with transposed-score layout ([tk, tq]) so softmax row-sums come
from a ones-column appended to V, normalizes late, and applies its row-slice
of the output projection. The host sums the 4 partial outputs per batch and
adds bout once.

Precision plan (tolerance 2e-2; lands ~5e-3):
- QKV projections: fp8e4m3 DoubleRow matmuls (0.5 cyc/row) with exact
  host-side residual splits x = x8 + xr, W = W8 + Wr; computes
  x8@W8 + xr@W8 + x8@Wr (drops only the ~0.001% xr@Wr term).
- Attention + out-projection: bf16 operands, fp32 PSUM accumulation.
- Softmax without max-subtraction (scores ~N(0,1)); exp on ACT with the
  1/sqrt(D) scale folded into Wq on the host.
"""

import numpy as np
import ml_dtypes

import concourse.bass as bass
import concourse.mybir as mybir
import concourse.tile as tile
from concourse import bacc
from concourse.bass_utils import run_bass_kernel_spmd

B, T, C, H, D = 2, 4096, 512, 8, 64
NCORES = 8
SCALE = 1.0 / np.sqrt(D)
# power-of-2 pre-scales that move the tiny projection weights out of
# fp8e4m3's denormal range; compensated exactly (exp scale, Wout/VSC)
QSC, KSC, VSC = 256.0, 32.0, 32.0

F32 = mybir.dt.float32
F32R = mybir.dt.float32r
BF16 = mybir.dt.bfloat16
F8 = mybir.dt.float8e4
DR = mybir.MatmulPerfMode.DoubleRow

TRACE = False
LAST_RESULT = None

_NC = None


def _build():
    nc = bacc.Bacc()

    # x^T fp8 value+residual pair: dim0 = 0..3 x8 chunks, 4..7 xr chunks
    xt = nc.declare_dram_parameter("xt", [8, 128, T], F8, isOutput=False)
    # q|k|v weight pairs, partition-first:
    # [128(c-rows), proj*24 + which(8|r)*4 + chunk, out-cols]
    wqkv = nc.declare_dram_parameter("wqkv", [128, 24, 128], F8, isOutput=False)
    wout = nc.declare_dram_parameter("wout", [128, 4, 128], BF16, isOutput=False)
    mask = nc.declare_dram_parameter("mask", [128, 128], BF16, isOutput=False)
    out_t = nc.declare_dram_parameter("out_t", [4, 128, T], BF16,
                                      isOutput=True)

    with tile.TileContext(nc) as tc:
        with (
            tc.tile_pool(name="w", bufs=1) as w,
            tc.tile_pool(name="sb", bufs=6) as sb,
            tc.tile_pool(name="sbN", bufs=8) as sbN,
            tc.tile_pool(name="sbA", bufs=8) as sbA,
            tc.tile_pool(name="psA", bufs=2, space="PSUM") as psA,
            tc.tile_pool(name="psO", bufs=2, space="PSUM") as psO,
            tc.tile_pool(name="psX", bufs=2, space="PSUM") as psX,
        ):
            # ---- persistent SBUF ----
            wqkv_s = w.tile([128, 24, 128], F8)
            wout_s = w.tile([128, 4, 128], BF16)
            mask_s = w.tile([128, 128], BF16)

            xt_s = w.tile([128, 8, T], F8)
            qt_s = w.tile([128, T], BF16)  # partitions: [h0 q-dims | h1]
            kt_s = w.tile([128, T], BF16)
            v_s = w.tile([128, 32, 130], BF16)  # per k-tile: [v_h0|1|v_h1|1]

            def _proj_mms(ps, wbase, g):
                """6 DoubleRow matmuls computing one 512-col group of a
                projection with residual correction, as single-mm pieces."""
                sl = bass.ts(g, 512)
                pieces = []
                seq = [(0, 0), (4, 0), (0, 1)]  # (x chunk base, w which)
                for ti, (xb, wi) in enumerate(seq):
                    for half in (0, 1):
                        st = ti == 0 and half == 0
                        sp = ti == 2 and half == 1
                        def mm(xb=xb, wi=wi, half=half, st=st, sp=sp):
                            c0 = wbase + wi * 4 + 2 * half
                            x0 = xb + 2 * half
                            nc.tensor.matmul(
                                ps,
                                wqkv_s[:, c0:c0 + 2, :],
                                xt_s[:, x0:x0 + 2, sl],
                                start=st, stop=sp, perf_mode=DR,
                            )
                        pieces.append(mm)
                return pieces

            def proj_qk(g, wbase, dst):
                """Q or K projection for column group g -> dst[:, g*512:...]."""
                pp = psX.tile([128, 512], F32, tag="x", name=f"qk{wbase}_{g}")
                pieces = _proj_mms(pp, wbase, g)
                if g == 0:
                    # two-half evac so chunk 0's first QK starts earlier
                    def evac_a():
                        nc.vector.tensor_copy(dst[:, 0:256], pp[:, 0:256])
                    def evac_b():
                        nc.vector.tensor_copy(dst[:, 256:512], pp[:, 256:512])
                    pieces.extend([evac_a, evac_b])
                    return pieces
                def evac():
                    nc.vector.tensor_copy(dst[:, bass.ts(g, 512)], pp)
                pieces.append(evac)
                return pieces

            def proj_v(g):
                """V for the 4 k-tiles of column group g, direct [t, (h d)]
                layout, one psum bank for all 4 tiles."""
                pv_ps = psX.tile([128, 4, 128], F32, tag="x", name=f"v_{g}")
                pieces = []
                for t4 in range(4):
                    tt = g * 4 + t4
                    seq = [(0, 0), (4, 0), (0, 1)]
                    for ti, (xb, wi) in enumerate(seq):
                        for half in (0, 1):
                            st = ti == 0 and half == 0
                            sp = ti == 2 and half == 1
                            def mm(xb=xb, wi=wi, half=half, st=st, sp=sp,
                                   t4=t4, tt=tt):
                                c0 = 16 + wi * 4 + 2 * half
                                x0 = xb + 2 * half
                                nc.tensor.matmul(
                                    pv_ps[:, t4, :],
                                    xt_s[:, x0:x0 + 2, bass.ts(tt, 128)],
                                    wqkv_s[:, c0:c0 + 2, :],
                                    start=st, stop=sp, perf_mode=DR,
                                )
                            pieces.append(mm)
                for h in (0, 1):
                    def evac(g=g, h=h):
                        nc.vector.tensor_copy(
                            v_s[:, 4 * g:4 * g + 4, h * 65:h * 65 + 64],
                            pv_ps[:, :, h * 64:h * 64 + 64],
                        )
                    pieces.append(evac)
                return pieces

            def queue_proj(g, dma=True):
                if dma:
                    sl = bass.ts(g, 512)
                    # one DMA per chunk; never on the scalar queue (DMAs on
                    # the ACT sequencer block exp dispatch).
                    nc.sync.dma_start(
                        out=xt_s[:, :, sl],
                        in_=xt[:, :, sl].rearrange("a p m -> p a m"))
                for late, cost, pieces in (
                    (0, 107.0, proj_qk(g, 0, qt_s)),
                    (1, 107.0, proj_qk(g, 8, kt_s)),
                    (1, 27.0, proj_v(g)),
                ):
                    for p in pieces:
                        proj_pending.append((g, late, cost, p))

            oc_tiles = {}

            def outproj_pieces(g, onorm_s, m):
                """Two pieces: the PE matmul, then the evac (+ one merged DMA
                per chunk after the last evac)."""
                st = {}
                def mm():
                    st["ps"] = psX.tile([128, 512], F32, tag="x",
                                        name=f"op{g}_{m}")
                    nc.tensor.matmul(
                        st["ps"], wout_s[:, m, :], onorm_s,
                        start=True, stop=True,
                    )
                def evac():
                    if g == 7:
                        # tail: split evacs over DVE + the now-idle ACT, and
                        # DMAs over two queues, so nothing serializes
                        oc_s = sb.tile([128, 512], BF16, tag="outc7",
                                       name=f"oc7_{m}")
                        if m % 2 == 0:
                            nc.vector.tensor_copy(oc_s, st.pop("ps"))
                        else:
                            nc.scalar.activation(
                                oc_s, st.pop("ps"),
                                mybir.ActivationFunctionType.Copy)
                        eng = nc.gpsimd if m % 2 == 0 else nc.sync
                        eng.dma_start(out=out_t[m][:, bass.ts(g, 512)],
                                      in_=oc_s)
                        return
                    if m == 0:
                        oc_tiles[g] = sb.tile([128, 4, 512], BF16, tag="outc",
                                              name=f"oc{g}")
                    oc_s = oc_tiles[g]
                    nc.vector.tensor_copy(oc_s[:, m, :], st.pop("ps"))
                    if m == 3:
                        nc.gpsimd.dma_start(
                            out=out_t[:, :, bass.ts(g, 512)].rearrange(
                                "a p m -> p a m"),
                            in_=oc_tiles.pop(g),
                        )
                return [(213.0, mm), (0.0, evac)]

            pv_pending = [None]
            deferred = []          # outproj pieces: (cost_ns, fn)
            proj_pending = []      # (g, late, cost_ns, fn)

            def flush_pv():
                if pv_pending[0] is not None:
                    pv_pending[0]()
                    pv_pending[0] = None

            def flush_proj(gmax, latemax):
                while proj_pending and (
                    proj_pending[0][0] < gmax
                    or (proj_pending[0][0] == gmax
                        and proj_pending[0][1] <= latemax)
                ):
                    proj_pending.pop(0)[3]()

            bucket = [0.0]

            def drip(use_deferred):
                """Emit PE filler, never exceeding the token bucket (strict:
                a piece only runs if fully paid for, so filler can never
                push the next slot's QK late)."""
                while proj_pending or (use_deferred and deferred):
                    if proj_pending:
                        _, _, cost, fn = proj_pending[0]
                        q = proj_pending
                    else:
                        cost, fn = deferred[0]
                        q = deferred
                    if cost > bucket[0]:
                        return
                    q.pop(0)
                    fn()
                    bucket[0] -= cost
                bucket[0] = 0.0

            # ---- startup ----
            # critical path first: wq, x(0), wk|wv; then mask/wout off-path
            sl0 = bass.ts(0, 512)
            nc.sync.dma_start(out=wqkv_s[:, 0:8, :], in_=wqkv[:, 0:8, :])
            nc.gpsimd.dma_start(
                out=xt_s[:, 0:4, sl0],
                in_=xt[0:4, :, sl0].rearrange("a p m -> p a m"))
            nc.sync.dma_start(out=wqkv_s[:, 8:24, :], in_=wqkv[:, 8:24, :])
            nc.sync.dma_start(
                out=xt_s[:, 4:8, sl0],
                in_=xt[4:8, :, sl0].rearrange("a p m -> p a m"))
            # touch Exp once so the ACT table loads during the startup DMAs
            warm_s = sb.tile([1, 1], F32, tag="warm")
            nc.vector.memset(warm_s, 0.25)
            nc.scalar.activation(warm_s, warm_s,
                                 mybir.ActivationFunctionType.Exp)
            nc.gpsimd.dma_start(out=mask_s, in_=mask[:])
            # ones-columns of V_aug (softmax row-sum trick)
            nc.vector.memset(
                v_s[:, :, 64:65].rearrange("p a b -> p (a b)"), 1.0)
            nc.vector.memset(
                v_s[:, :, 129:130].rearrange("p a b -> p (a b)"), 1.0)
            nc.gpsimd.dma_start(out=wout_s, in_=wout[:])

            queue_proj(0, dma=False)
            # q(0), k(0) must be ready before the first score group; v(0)
            # drains under the first exp (forced before the first PV below)
            flush_proj(0, 0)
            for _ in range(8):  # k(0): 6 matmuls + 2 evac halves
                proj_pending.pop(0)[3]()

            # ---- global pipelined stream of score-groups ----
            slots = []
            for g in range(8):
                njs = 4 * g + 4
                jgs = [list(range(j0, min(j0 + 2, njs)))
                       for j0 in range(0, njs, 2)]
                for h in (0, 1):
                    for gi, js in enumerate(jgs):
                        slots.append((g, h, gi, js, gi == len(jgs) - 1))
            proj_deadline = {}
            chunk_start = {}
            for idx, (g, h, gi, js, last) in enumerate(slots):
                if h == 0 and gi == 2 * g:
                    proj_deadline[g] = idx
                if h == 0 and gi == 0:
                    chunk_start[g] = idx
            total_slots = len(slots)

            onorms = {}
            seg_o = {}
            oc7 = {}

            def norm_cols(o_ps, hb, onorm_s, c0, c1, tail=False):
                """Softmax normalization for cols [c0:c1)."""
                ww = c1 - c0
                rec_s = sb.tile([1, 512], F32R, tag="rec")
                with nc.allow_low_precision(reason="fp32r recip"):
                    nc.vector.reciprocal(rec_s[:, 0:ww], o_ps[64:65, c0:c1])
                bc_sb = sb.tile([64, 512], F32R, tag="bc")
                nc.gpsimd.partition_broadcast(bc_sb[:, 0:ww], rec_s[:, 0:ww])
                nc.vector.tensor_tensor(
                    onorm_s[hb:hb + 64, c0:c1], o_ps[0:64, c0:c1],
                    bc_sb[:, 0:ww], mybir.AluOpType.mult,
                )

            def tail_ops(hh, onorm_s):
                """Half-width out-projection for chunk 7, split over engines.
                All matmuls first, then copies, to keep the DVE wait queue
                shallow for the second half's reciprocal."""
                c0 = 256 * hh
                if hh == 0:
                    oc7[0] = sb.tile([128, 2, 512], BF16, tag="outc7",
                                     name="oc7_01")
                    oc7[1] = sb.tile([128, 2, 512], BF16, tag="outc7",
                                     name="oc7_23")
                tiles = []
                for m in range(4):
                    ps = psX.tile([128, 256], F32, tag="x",
                                  name=f"tl{hh}_{m}")
                    nc.tensor.matmul(
                        ps, wout_s[:, m, :], onorm_s[:, c0:c0 + 256],
                        start=True, stop=True,
                    )
                    tiles.append(ps)
                for m in range(4):
                    oc_s = oc7[m // 2][:, m % 2, :]
                    # half 0's copies all on ACT so DVE's wait window stays
                    # clear for the second half's reciprocal; half 1 splits
                    if hh == 0 or m % 2 == 1:
                        nc.scalar.activation(
                            oc_s[:, c0:c0 + 256], tiles[m],
                            mybir.ActivationFunctionType.Copy)
                    else:
                        nc.vector.tensor_copy(oc_s[:, c0:c0 + 256], tiles[m])
                    if hh == 1 and m % 2 == 1:
                        # two merged DMAs on the hw queue
                        nc.sync.dma_start(
                            out=out_t[m - 1:m + 1, :, bass.ts(7, 512)]
                            .rearrange("a p m -> p a m"),
                            in_=oc7[m // 2],
                        )

            sc_tiles = {}

            def emit_qk(idx):
                """Chunk/segment bookkeeping + QK matmuls for slot idx.
                Called one slot ahead of the slot's exp so the exp input is
                always a full slot old and ACT never waits on PE."""
                g, h, gi, js, last = slots[idx]
                hb = h * 64
                if h == 0 and gi == 0:
                    if g < 7:
                        queue_proj(g + 1)
                    onorms[g] = sbN.tile([128, 512], BF16, tag="onorm",
                                         name=f"onorm{g}")
                    flush_proj(g, 0)   # q(g) ready
                if gi == 0:
                    seg_o[(g, h)] = psO.tile([65, 512], F32, tag="o",
                                             name=f"o{g}_{h}")
                if h == 0 and gi == (1 if g == 0 else 2 * g):
                    flush_proj(g, 1)   # k(g), v(g) ready before diagonal/PV
                n = len(js)
                sc_ps = psA.tile([128, 1024], F32, tag="bigA",
                                 name=f"sc{idx}")
                offs = [max(0, (j - 4 * g) * 128) for j in js]
                starts = [offs[0]] + [512] * (n - 1)
                ends = [starts[i] + 512 - offs[i] for i in range(n)]
                for i2, j in enumerate(js):
                    nc.tensor.matmul(
                        sc_ps[:, starts[i2]:ends[i2]],
                        kt_s[hb:hb + 64, bass.ts(j, 128)],
                        qt_s[hb:hb + 64, g * 512 + offs[i2]:(g + 1) * 512],
                        start=True, stop=True,
                    )
                sc_tiles[idx] = (sc_ps, offs, starts, ends)

            emit_qk(0)
            for idx, (g, h, gi, js, last) in enumerate(slots):
                njs = 4 * g + 4
                hb = h * 64
                if idx + 1 < total_slots:
                    emit_qk(idx + 1)
                o_ps = seg_o[(g, h)]
                onorm_s = onorms[g]
                sc_ps, offs, starts, ends = sc_tiles.pop(idx)

                at_s = sbA.tile([128, 1024], BF16, tag="attn")
                nc.scalar.activation(
                    at_s[:, starts[0]:ends[-1]], sc_ps[:, starts[0]:ends[-1]],
                    mybir.ActivationFunctionType.Exp,
                    scale=1.0 / (QSC * KSC),
                )
                flush_pv()
                # pace pending PE work against its deadline; finish all
                # deferred outprojs by chunk 6 so late chunks (where ACT is
                # the pacer) run pure attention
                use_deferred = g >= 4
                budget = 0.0
                if proj_pending:
                    dl = proj_deadline.get(proj_pending[-1][0], idx + 1)
                    budget += (sum(p[2] for p in proj_pending)
                               / max(dl - idx, 1))
                if use_deferred and deferred:
                    budget += (sum(c for c, _ in deferred)
                               / max(chunk_start[7] - idx, 1))
                bucket[0] += budget
                drip(use_deferred)

                def pv(js=js, offs=offs, starts=starts, ends=ends,
                       at_s=at_s, o_ps=o_ps, h=h, hb=hb, njs=njs, g=g,
                       gi=gi, last=last, onorm_s=onorm_s):
                    for i2, j in enumerate(js):
                        d = j - 4 * g
                        if d >= 0:
                            # triangular mask on the first 128 valid cols
                            s0 = starts[i2]
                            nc.vector.tensor_tensor(
                                at_s[:, s0:s0 + 128],
                                at_s[:, s0:s0 + 128],
                                mask_s,
                                mybir.AluOpType.mult,
                            )
                        nc.tensor.matmul(
                            o_ps[0:65, offs[i2]:512],
                            v_s[:, j, h * 65:(h + 1) * 65],
                            at_s[:, starts[i2]:ends[i2]],
                            start=(j == 0), stop=(j == njs - 1),
                        )
                    if g == 7 and h == 1 and gi == 2 * g:
                        # tail: cols [0:256) are final one PV-group early
                        norm_cols(o_ps, hb, onorm_s, 0, 256)
                        tail_ops(0, onorm_s)
                    elif g == 7 and h == 1 and last:
                        norm_cols(o_ps, hb, onorm_s, 256, 512)
                        tail_ops(1, onorm_s)
                    elif last:
                        norm_cols(o_ps, hb, onorm_s, 0, 512)
                pv_pending[0] = pv

                if h == 1 and last and g < 7:
                    for m in range(4):
                        deferred.extend(outproj_pieces(g, onorm_s, m))
            flush_pv()
            for _, fn in deferred:
                fn()
    nc.compile()
    return nc


def _split8(a):
    """fp8e4m3 value + residual split (value + residual ~= a to ~0.1%)."""
    a8 = a.astype(ml_dtypes.float8_e4m3)
    ar = (a - a8.astype(np.float32)).astype(ml_dtypes.float8_e4m3)
    return a8, ar


def _pack_inputs(x, Wqkv, bqkv, Wout, bout):
    mask_ut = np.triu(np.ones((128, 128), dtype=np.float32))
    in_maps = []
    for c in range(NCORES):
        b = c // 4
        h0 = 2 * (c % 4)
        xtb = np.ascontiguousarray(x[b].T).reshape(4, 128, T)
        xt8, xtr = _split8(xtb)
        xt_pair = np.ascontiguousarray(np.concatenate([xt8, xtr], axis=0))
        wq_f = Wqkv[:, h0 * 64:h0 * 64 + 128] * (SCALE * QSC)
        wk_f = Wqkv[:, 512 + h0 * 64:512 + h0 * 64 + 128] * KSC
        wv_f = Wqkv[:, 1024 + h0 * 64:1024 + h0 * 64 + 128] * VSC
        packs = []
        for wf in (wq_f, wk_f, wv_f):
            w8, wr = _split8(np.ascontiguousarray(wf.reshape(4, 128, 128)))
            # [128(p), 2(8|r)*4(chunk), 128(m)]
            packs.append(np.stack([w8, wr], axis=0).reshape(8, 128, 128)
                         .transpose(1, 0, 2))
        wqkv = np.ascontiguousarray(np.concatenate(packs, axis=1))
        wout_c = np.ascontiguousarray(
            (Wout[h0 * 64:h0 * 64 + 128, :] / VSC).reshape(128, 4, 128))
        in_maps.append({
            "xt": xt_pair, "wqkv": wqkv,
            "wout": wout_c.astype(ml_dtypes.bfloat16),
            "mask": mask_ut.astype(ml_dtypes.bfloat16),
        })
    return in_maps


def kernel(x, Wqkv, bqkv, Wout, bout):
    global _NC, LAST_RESULT
    x = np.asarray(x, dtype=np.float32)
    Wqkv = np.asarray(Wqkv, dtype=np.float32)
    bqkv = np.asarray(bqkv, dtype=np.float32)
    Wout = np.asarray(Wout, dtype=np.float32)
    bout = np.asarray(bout, dtype=np.float32)
    assert np.all(bqkv == 0.0), "kernel assumes zero qkv bias (spec: zeros)"

    if _NC is None:
        _NC = _build()
    in_maps = _pack_inputs(x, Wqkv, bqkv, Wout, bout)
    res = run_bass_kernel_spmd(_NC, in_maps, list(range(NCORES)), trace=TRACE)
    LAST_RESULT = res
    out = np.zeros((B, T, C), dtype=np.float32)
    for c in range(NCORES):
        part = res.results[c]["out_t"].astype(np.float32).reshape(C, T)
        out[c // 4] += part.T
    out += bout
    return out
